# revision 40
# baseline (speedup 1.0000x reference)
"""2-layer GCN encoder on 8 Trainium2 NeuronCores (Bass/Tile).

Strategy: nodes sharded across 8 cores; edges partitioned by destination
node; scatter-add done locally per dst shard via one-hot matmuls on the
tensor engine.

Math: GCNConv's symmetric norm factorizes: norm(e) = dinv[src]*dinv[dst].
With g = dinv * (x @ W), the aggregation is out[d] = dinv[d]*(g[d] +
sum_{e:dst=d} g[src(e)]) + b (the g[d] term is the self loop, applied from
local SBUF data via an identity matmul -- never gathered).

Perf structure:
  - the layer-1 table g1 = dinv*x is computed LOCALLY on every core from
    the full x input (x is replicated; only a bf16 cast happens on host),
    so no layer-1 AllGather exists and gathers start at ~40us
  - layer-2 table g2 is AllGathered in 3 stripes, issued as soon as each
    stripe's windows are produced, overlapping the layer-1 tail
  - three table stripes -> 3 gather calls per window, rotated over the 4
    SWDGE queues (~0.75 calls/queue/window): the per-queue descriptor ring
    only holds about one call, so per-queue call cadence (~6us) is the hard
    limit and fewer calls per window is the lever
  - per-core trailing -1 index padding + runtime num_idxs_reg (loaded from
    SBUF into rotating gpsimd registers) trims ~11% of gather rows
  - PSUM->SBUF copies ride the scalar engine to keep DVE free for the
    one-hot S builds
"""

import sys

for _p in ("/opt/trn_rl_repo", "/opt/trn_rl_repo/concourse"):
    if _p not in sys.path:
        sys.path.insert(0, _p)

import numpy as np

import concourse.bacc as bacc
import concourse.mybir as mybir
import concourse.tile as tile
from concourse.bass_utils import run_bass_kernel_spmd
from concourse.library_config import mlp as _mlp_lib

F32 = mybir.dt.float32
BF16 = mybir.dt.bfloat16
I16 = mybir.dt.int16
I32 = mybir.dt.int32
AF = mybir.ActivationFunctionType
ALU = mybir.AluOpType

C = 8          # cores
P = 128        # partitions / window size
SWIN = (20, 20, 9)        # windows per table stripe (sum = NT)
NSW = len(SWIN)
CALLCH = 8                # max chunks (of 128 rows) per dma_gather call
PAYBUFS = (6, 6, 5)       # layer-1 payload ring depth per stripe
PAYBUFS2 = (6, 6, 5)      # layer-2 payload ring depth per stripe
TW = 8                    # table windows per g1-production piece
TRIM = True    # pad gather idx streams with -1 (ucode trims per core)
W0NT = 8       # first windows stay untrimmed (payload-buffer init source)


def _ceil(a, b):
    return (a + b - 1) // b


class Plan:
    """Static structure shared by all cores (program shape)."""

    def __init__(self, N, E, DIN, DH, DOUT):
        self.N, self.E = N, E
        self.DIN, self.DH, self.DOUT = DIN, DH, DOUT
        self.NLOC = N // C
        self.NT = _ceil(self.NLOC, P)
        self.NLOC_PAD = self.NT * P
        assert sum(SWIN) == self.NT
        self.SW = [w * P for w in SWIN]          # stripe widths (locs)
        self.SB = np.concatenate([[0], np.cumsum(self.SW)])  # bounds
        self.STBL = [C * w for w in self.SW]     # rows per stripe table
        assert all(t <= 32768 for t in self.STBL)
        self.NTBL = sum(self.STBL) // P          # total table windows
        self.CH = None        # [NT, NSW] common chunk counts
        self.CHS = None       # [NSW] max chunks per call (pay tile dim)
        self.offs = None      # [NSW][NT] chunk offsets within stream
        self.woff = None      # [NT] dstrel column offset of window w
        self.LS = None        # [NSW] stream rows
        self.TOTCH = None
        self.CHMAX = None     # max chunks per window (iota/S dim)
        self.calls = None     # [NSW] list of (chunk0, nch, window)
        self.NCALL = None

    def key(self):
        return (self.N, self.E, self.CH.tobytes())


def preprocess(x, edge_index, W1, b1, W2, b2):
    """Host-side sharding: integer index metadata only (no float compute on
    the feature data beyond dtype casts / layout of the given tensors)."""
    x = np.asarray(x)
    N, DIN = x.shape
    E = edge_index.shape[1]
    DH = W1.shape[1]
    DOUT = W2.shape[1]
    plan = Plan(N, E, DIN, DH, DOUT)
    NLOC, NT = plan.NLOC, plan.NT

    src = np.asarray(edge_index[0], dtype=np.int64)
    dst = np.asarray(edge_index[1], dtype=np.int64)

    # degree with self loop -- integer graph metadata
    deg = (np.bincount(dst, minlength=N) + 1).astype(np.float32)

    co = dst // NLOC                     # owning core (by dst)
    dl = dst - co * NLOC
    w_of = dl // P
    rel = (dl % P).astype(np.int16)
    sc = src // NLOC
    sl = src - sc * NLOC
    stripe = np.searchsorted(plan.SB, sl, side="right") - 1   # 0..NSW-1
    rowidx = (sc * np.array(plan.SW)[stripe] + sl
              - plan.SB[stripe]).astype(np.int16)

    # group edges by (core, window, stripe)
    gid = (co * NT + w_of) * NSW + stripe
    order = np.argsort(gid, kind="stable")
    gid_s = gid[order]
    row_s = rowidx[order]
    rel_s = rel[order]
    n_groups = C * NT * NSW
    counts = np.bincount(gid_s, minlength=n_groups).reshape(C, NT, NSW)
    starts = np.concatenate([[0], np.cumsum(counts.reshape(-1))[:-1]]).reshape(
        C, NT, NSW)

    # common chunk structure = max over cores
    CH = np.ceil(counts.max(axis=0) / P).astype(np.int64)      # [NT, NSW]
    plan.CH = CH
    plan.offs = [np.concatenate([[0], np.cumsum(CH[:, s])[:-1]]) for s in range(NSW)]
    plan.LS = [int(CH[:, s].sum()) * P for s in range(NSW)]
    chw = CH.sum(axis=1)
    plan.woff = np.concatenate([[0], np.cumsum(chw)[:-1]])
    plan.TOTCH = int(chw.sum())
    plan.CHMAX = int(chw.max())

    # one call per (window, stripe) group, split at CALLCH chunks (ucode max
    # 1024 rows); per-core trailing padding is -1 so the gather ucode
    # truncates it, with num_idxs_reg carrying the per-core valid count so
    # decode-side ring accounting matches what the ucode writes
    plan.calls = []
    call_defs = []   # [(s, w, c0_chunk, nch, k0_rows)]
    for s in range(NSW):
        calls = []
        for w in range(NT):
            nch = int(CH[w, s])
            k = 0
            while k < nch:
                n = min(CALLCH, nch - k)
                c0 = int(plan.offs[s][w]) + k
                calls.append((c0, n, w))
                call_defs.append((s, w, c0, n, k * P))
                k += n
        plan.calls.append(calls)
    plan.NCALL = len(call_defs)
    plan.CHS = [max(n for (_c, n, _w) in plan.calls[s]) for s in range(NSW)]

    def wrap_idx(a):
        # [n] -> [16, n//16] column-major wrap, replicated x8 -> [128, n//16]
        n = a.shape[0]
        w = a.reshape(n // 16, 16).T
        return np.ascontiguousarray(np.tile(w, (8, 1)))

    bf16 = mybir.dt.np(BF16)

    # full-graph table layout (shared by all cores): stripe s rows are
    # [core, loc-within-stripe]; plus deg in the piece-major layout the
    # device-side scale pass reads ([p, t] = row r0 + p*TW + t)
    NLP = plan.NLOC_PAD
    xpad_all = np.zeros((C * NLP, DIN), np.float32)
    degpad_all = np.ones((C * NLP,), np.float32)
    for k in range(C):
        xpad_all[k * NLP:k * NLP + NLOC] = x[k * NLOC:(k + 1) * NLOC]
        degpad_all[k * NLP:k * NLP + NLOC] = deg[k * NLOC:(k + 1) * NLOC]
    xtbl = []
    degT_cols = []
    for s in range(NSW):
        rows = np.concatenate([
            np.arange(plan.SB[s], plan.SB[s + 1]) + k * NLP for k in range(C)])
        xtbl.append(np.ascontiguousarray(xpad_all[rows].astype(bf16)))
        dg = degpad_all[rows]
        npieces = plan.STBL[s] // (TW * P)
        for i in range(npieces):
            blk = dg[i * TW * P:(i + 1) * TW * P].reshape(P, TW)
            degT_cols.append(blk)
    degT = np.ascontiguousarray(np.concatenate(degT_cols, axis=1))  # [P, NTBL]

    in_maps = []
    for c in range(C):
        lo = c * NLOC
        # x, pre-transposed to [128, NT*DIN]: col w*DIN+f, row p = node w*P+p
        xpad = np.zeros((plan.NLOC_PAD, DIN), np.float32)
        xpad[:NLOC] = x[lo:lo + NLOC]
        xshT = np.ascontiguousarray(
            xpad.reshape(NT, P, DIN).transpose(1, 0, 2).reshape(P, NT * DIN))

        degf = np.ones((plan.NLOC_PAD,), np.float32)
        degf[:NLOC] = deg[lo:lo + NLOC]
        degf = np.ascontiguousarray(degf.reshape(NT, P).T)   # [P, NT]

        fill = -1 if TRIM else 0
        idxs = [np.full((plan.LS[s],), fill, np.int16) for s in range(NSW)]
        drel = np.full((plan.TOTCH * P,), -1, np.int16)
        for w in range(NT):
            for s in range(NSW):
                n = counts[c, w, s]
                st = starts[c, w, s]
                p0 = plan.offs[s][w] * P
                idxs[s][p0:p0 + n] = row_s[st:st + n]
                col0 = plan.woff[w] + int(CH[w, :s].sum())
                drel[col0 * P:col0 * P + n] = rel_s[st:st + n]

        # per-core valid count per call; interior (non-final) calls of a
        # split group keep 0-idx padding so only the final tail is -1
        gcnt = np.zeros((plan.NCALL,), np.int32)
        for ci, (s, w, c0, n, k0) in enumerate(call_defs):
            cnt = int(counts[c, w, s])
            gcnt[ci] = max(0, min(cnt - k0, n * P))
            if k0 + n * P < int(CH[w, s]) * P and cnt - k0 < n * P:
                # interior call of a split group: fill its tail with 0-idx
                p0 = c0 * P + max(0, cnt - k0)
                idxs[s][p0:(c0 + n) * P] = 0

        im = {
            "xshT": xshT,
            "W1": np.asarray(W1, np.float32).astype(bf16),
            "W2": np.asarray(W2, np.float32).astype(bf16),
            "b1r": np.tile(np.asarray(b1, np.float32)[None, :], (P, 1)),
            "b2r": np.tile(np.asarray(b2, np.float32)[None, :], (P, 1)),
            "degf": degf,
            "degT": degT,
            "dstrel": np.ascontiguousarray(drel.reshape(plan.TOTCH, P).T),
            "gcnt": np.tile(gcnt[None, :], (P, 1)),
        }
        for s in range(NSW):
            im[f"idx{s}"] = wrap_idx(idxs[s])
            im[f"xtbl{s}"] = xtbl[s]
        in_maps.append(im)
    return plan, in_maps


def build(plan: Plan):
    DIN, DH, DOUT = plan.DIN, plan.DH, plan.DOUT
    NT = plan.NT
    NLOC = plan.NLOC
    CH, offs, woff = plan.CH, plan.offs, plan.woff
    CHMAX = plan.CHMAX
    LOOKAHEAD = [max(1, b - 2) for b in PAYBUFS]
    LOOKAHEAD2 = [max(1, b - 2) for b in PAYBUFS2]

    nc = bacc.Bacc("TRN2", target_bir_lowering=False, debug=False, num_devices=C,
                   dynamic_dma_scratch_size=32768, num_swdge_queues=4)

    xshT = nc.dram_tensor("xshT", [P, NT * DIN], F32, kind="ExternalInput")
    W1 = nc.dram_tensor("W1", [DIN, DH], BF16, kind="ExternalInput")
    W2 = nc.dram_tensor("W2", [DH, DOUT], BF16, kind="ExternalInput")
    b1r = nc.dram_tensor("b1r", [P, DH], F32, kind="ExternalInput")
    b2r = nc.dram_tensor("b2r", [P, DOUT], F32, kind="ExternalInput")
    degf = nc.dram_tensor("degf", [P, NT], F32, kind="ExternalInput")
    degT = nc.dram_tensor("degT", [P, plan.NTBL], F32, kind="ExternalInput")
    xtbl_d = [nc.dram_tensor(f"xtbl{s}", [plan.STBL[s], DIN], BF16,
                             kind="ExternalInput") for s in range(NSW)]
    idx_d = [nc.dram_tensor(f"idx{s}", [P, plan.LS[s] // 16], I16,
                            kind="ExternalInput") for s in range(NSW)]
    dstrel = nc.dram_tensor("dstrel", [P, plan.TOTCH], I16, kind="ExternalInput")
    gcnt = nc.dram_tensor("gcnt", [P, plan.NCALL], I32, kind="ExternalInput")
    out = nc.dram_tensor("out", [NLOC, DOUT], F32, kind="ExternalOutput")

    g1_tbl = [nc.dram_tensor(f"g1_tbl{s}", [plan.STBL[s], DIN], BF16)
              for s in range(NSW)]
    g2_in = [nc.dram_tensor(f"g2_in{s}", [plan.SW[s], DOUT], BF16)
             for s in range(NSW)]
    g2_tbl = [nc.dram_tensor(f"g2_tbl{s}", [plan.STBL[s], DOUT], BF16,
                             addr_space="Shared") for s in range(NSW)]

    def all_gather(src, dsts):
        nc.gpsimd.collective_compute(
            "AllGather", ALU.bypass,
            replica_groups=[list(range(C))],
            ins=[src.ap().opt()],
            outs=[dsts.ap().opt()])

    with tile.TileContext(nc) as tc:
        with tc.tile_pool(name="const", bufs=1) as cpool, \
             tc.tile_pool(name="sbuild", bufs=3) as spool, \
             tc.tile_pool(name="epi", bufs=3) as epool, \
             tc.tile_pool(name="pay1_0", bufs=PAYBUFS[0]) as pp1a, \
             tc.tile_pool(name="pay1_1", bufs=PAYBUFS[1]) as pp1b, \
             tc.tile_pool(name="pay1_2", bufs=PAYBUFS[2]) as pp1c, \
             tc.tile_pool(name="pay2_0", bufs=PAYBUFS2[0]) as pp2a, \
             tc.tile_pool(name="pay2_1", bufs=PAYBUFS2[1]) as pp2b, \
             tc.tile_pool(name="pay2_2", bufs=PAYBUFS2[2]) as pp2c:

            paypools1 = (pp1a, pp1b, pp1c)
            paypools2 = (pp2a, pp2b, pp2c)

            nc.gpsimd.load_library(_mlp_lib)

            # zero the payload rings once: trailing-trimmed gather calls
            # leave tail slots unwritten, and uninitialized SBUF could hold
            # NaN patterns that would poison 0*NaN in the PE accumulate
            def prefill(pools, bufs, layer, dim):
                for s in range(NSW):
                    for _ in range(bufs[s]):
                        t = pools[s].tile([P, plan.CHS[s], dim], BF16,
                                          tag=f"pay{layer}_{s}")
                        nc.vector.memset(t[:, :, :], 0)
            prefill(paypools1, PAYBUFS, 1, DIN)
            prefill(paypools2, PAYBUFS2, 2, DOUT)

            # ---- degrees -> dinv (local windows and full table) ----
            deg_sb = cpool.tile([P, NT], F32, tag="deg")
            nc.sync.dma_start(deg_sb[:, :], degf[:, :])
            sq_sb = cpool.tile([P, NT], F32, tag="sqdeg")
            nc.scalar.activation(sq_sb[:, :], deg_sb[:, :], AF.Sqrt)
            dinv_sb = cpool.tile([P, NT], F32, tag="dinv")
            nc.vector.reciprocal(dinv_sb[:, :], sq_sb[:, :])

            degT_sb = cpool.tile([P, plan.NTBL], F32, tag="degT")
            nc.sync.dma_start(degT_sb[:, :], degT[:, :])
            sqT_sb = cpool.tile([P, plan.NTBL], F32, tag="sqT")
            nc.scalar.activation(sqT_sb[:, :], degT_sb[:, :], AF.Sqrt)
            dinvT_sb = cpool.tile([P, plan.NTBL], F32, tag="dinvT")
            nc.vector.reciprocal(dinvT_sb[:, :], sqT_sb[:, :])

            # gather metadata loads + iota/identity
            dstrel_sb = cpool.tile([P, plan.TOTCH], I16, tag="dstrel")
            nc.sync.dma_start(dstrel_sb[:, :], dstrel[:, :])
            gcnt_sb = cpool.tile([P, plan.NCALL], I32, tag="gcnt")
            nc.sync.dma_start(gcnt_sb[:, :], gcnt[:, :])
            scall0 = np.concatenate(
                [[0], np.cumsum([len(c) for c in plan.calls])]).astype(int)
            NREGS = 16
            cnt_regs = [nc.gpsimd.alloc_register(f"gcnt_r{i}")
                        for i in range(NREGS)] if TRIM else None
            regrot = [0]
            qrot = [0]
            idx_sb = []
            for s in range(NSW):
                t = cpool.tile([P, plan.LS[s] // 16], I16, tag=f"idx{s}",
                               name=f"idx{s}")
                nc.sync.dma_start(t[:, :], idx_d[s][:, :])
                idx_sb.append(t)

            iota_sb = cpool.tile([P, CHMAX, P], I16, tag="iota")
            nc.gpsimd.iota(iota_sb[:, :, :], pattern=[[0, CHMAX], [1, P]],
                           base=0, channel_multiplier=0)
            iota_p = cpool.tile([P, P], I16, tag="iota_p")
            nc.gpsimd.iota(iota_p[:, :], pattern=[[0, P]], base=0,
                           channel_multiplier=1)
            ident = cpool.tile([P, P], BF16, tag="ident")
            nc.vector.tensor_tensor(ident[:, :], iota_sb[:, 0, :], iota_p[:, :],
                                    ALU.is_equal)

            # weights / biases (already bf16 from host)
            W1_sb = cpool.tile([P, DH], BF16, tag="W1")
            nc.sync.dma_start(W1_sb[:, :], W1[:, :])
            W2_sb = []
            for k in range(DH // P):
                t = cpool.tile([P, DOUT], BF16, tag=f"W2_{k}", name=f"W2_{k}")
                nc.sync.dma_start(t[:, :], W2[k * P:(k + 1) * P, :])
                W2_sb.append(t)
            b1_sb = cpool.tile([P, DH], F32, tag="b1")
            nc.sync.dma_start(b1_sb[:, :], b1r[:, :])
            b2_sb = cpool.tile([P, DOUT], F32, tag="b2")
            nc.sync.dma_start(b2_sb[:, :], b2r[:, :])

            # ---- local g1 table: load x table, scale by dinv, store ----
            # (replaces the layer-1 AllGather: x is replicated, so every core
            # builds the full table itself; pieces stream stripe by stripe so
            # stripe-0 gathers can start almost immediately)
            gpc = 0   # global piece counter (degT column base = gpc*TW)
            with tc.tile_pool(name="xt", bufs=3) as xtpool:
                for s in range(NSW):
                    npieces = plan.STBL[s] // (TW * P)
                    for i in range(npieces):
                        r0 = i * TW * P
                        x_t = xtpool.tile([P, TW, DIN], BF16, tag="xt",
                                          name=f"xt_{s}_{i}")
                        nc.sync.dma_start(x_t[:, :, :],
                                          xtbl_d[s][r0:r0 + TW * P, :])
                        g_t = xtpool.tile([P, TW, DIN], BF16, tag="gt",
                                          name=f"gt_{s}_{i}")
                        nc.vector.tensor_tensor(
                            g_t[:, :, :], x_t[:, :, :],
                            dinvT_sb[:, gpc * TW:(gpc + 1) * TW]
                            .unsqueeze(-1).broadcast_to((P, TW, DIN)),
                            ALU.mult)
                        nc.sync.dma_start(g1_tbl[s][r0:r0 + TW * P, :],
                                          g_t[:, :, :])
                        gpc += 1

            # ---- local rows g1 (self loops), from the pre-transposed x ----
            g1k = tc.tile_pool(name="g1keep", bufs=1)
            g1kpool = g1k.__enter__()
            g1_sb = g1kpool.tile([P, NT, DIN], BF16, tag="g1")
            with tc.tile_pool(name="xg", bufs=1) as xgpool:
                for wp in range(0, NT, 7):
                    np_ = min(7, NT - wp)
                    x_sb = xgpool.tile([P, 7, DIN], F32, tag="x",
                                       name=f"x_{wp}")
                    nc.sync.dma_start(x_sb[:, :np_, :],
                                      xshT[:, wp * DIN:(wp + np_) * DIN])
                    nc.vector.tensor_tensor(
                        g1_sb[:, wp:wp + np_, :], x_sb[:, :np_, :],
                        dinv_sb[:, wp:wp + np_].unsqueeze(-1)
                        .broadcast_to((P, np_, DIN)),
                        ALU.mult)

            # g2 rows stay resident for the layer-2 self-loop contribution
            g2_sb = cpool.tile([P, NT, DOUT], BF16, tag="g2keep")

            # ---- a generic gather/consume pass -------------------------------
            def run_pass(layer, tbls, dim, consume_fn, paypools, look, bufs,
                         agpts=()):
                chunk_tile = [dict() for _ in range(NSW)]
                heads = [0] * NSW

                def issue_next(s):
                    gi = int(scall0[s]) + heads[s]
                    c0, nch, _fw = plan.calls[s][heads[s]]
                    heads[s] += 1
                    pay = paypools[s].tile([P, plan.CHS[s], dim], BF16,
                                           tag=f"pay{layer}_{s}",
                                           name=f"pay{layer}_{s}_{c0}")
                    n = nch * P
                    if TRIM:
                        nreg = cnt_regs[regrot[0] % NREGS]
                        regrot[0] += 1
                        nc.gpsimd.reg_load(nreg, gcnt_sb[0:1, gi:gi + 1])
                    else:
                        nreg = n
                    nc.gpsimd.dma_gather(
                        pay[:, :nch, :], tbls[s][:, :],
                        idx_sb[s][:, c0 * 8:(c0 + nch) * 8],
                        n, nreg, dim, queue_num=qrot[0] % 4)
                    qrot[0] += 1
                    for j in range(nch):
                        chunk_tile[s][c0 + j] = (pay, j)

                # prologue bands: fill each stripe's ring in stripe order so
                # no dispatch waits on a later stripe's table/AG
                for s in range(NSW):
                    for _ in range(min(bufs[s], len(plan.calls[s]))):
                        issue_next(s)

                def issue_due(w):
                    while True:
                        best = None
                        for s in range(NSW):
                            if heads[s] < len(plan.calls[s]):
                                fw = plan.calls[s][heads[s]][2]
                                if fw <= w + look[s] and \
                                        (best is None or fw < best[1]):
                                    best = (s, fw)
                        if best is None:
                            return
                        issue_next(best[0])

                for w in range(NT):
                    issue_due(w)
                    chw = int(CH[w, :].sum())
                    S = spool.tile([P, CHMAX, P], BF16, tag="S")
                    if chw:
                        nc.vector.tensor_tensor(
                            S[:, :chw, :],
                            dstrel_sb[:, woff[w]:woff[w] + chw]
                            .unsqueeze(-1).broadcast_to((P, chw, P)),
                            iota_sb[:, :chw, :], ALU.is_equal)
                    ps = consume_fn.psum(w)
                    flip = (layer == 1)
                    # self contribution from local rows (never gathered).
                    # Layer 1 computes the TRANSPOSED aggregate [feat, dst]
                    # (payload stationary, one-hot moving; same PE cycles) so
                    # the W1 GEMM consumes it without a transpose.
                    sr = consume_fn.selfrows(w)
                    nc.tensor.matmul(ps[:, :],
                                     sr if flip else ident[:, :],
                                     ident[:, :] if flip else sr,
                                     start=True, stop=(chw == 0))
                    j = 0
                    for s in range(NSW):
                        for q in range(offs[s][w], offs[s][w] + int(CH[w, s])):
                            pay, slot = chunk_tile[s][q]
                            a = pay[:, slot, :] if flip else S[:, j, :]
                            b = S[:, j, :] if flip else pay[:, slot, :]
                            nc.tensor.matmul(ps[:, :], a, b,
                                             start=False, stop=(j == chw - 1))
                            j += 1
                    consume_fn.epilogue(w, ps)
                    if w in agpts:
                        agpts[w]()

            # ---- layer 1 consume + layer-2 producer --------------------------
            with tc.tile_pool(name="ps1", bufs=2, space="PSUM") as pspool1, \
                 tc.tile_pool(name="pt1", bufs=2, space="PSUM") as ptpool1:

                class L1:
                    @staticmethod
                    def psum(w):
                        return pspool1.tile([P, DIN], F32, tag="agg1",
                                            name=f"agg1_{w}")

                    @staticmethod
                    def selfrows(w):
                        return g1_sb[:, w, :]

                    @staticmethod
                    def epilogue(w, ps):
                        aggT = epool.tile([P, DIN], BF16, tag="aggT")
                        nc.scalar.activation(aggT[:, :], ps[:, :], AF.Copy)
                        ps1 = pspool1.tile([P, DH], F32, tag="gemm1",
                                           name=f"gemm1_{w}")
                        nc.tensor.matmul(ps1[:, :], aggT[:, :], W1_sb[:, :],
                                         start=True, stop=True)
                        v = epool.tile([P, DH], F32, tag="v1")
                        nc.vector.scalar_tensor_tensor(v[:, :], ps1[:, :],
                                                       dinv_sb[:, w:w + 1],
                                                       b1_sb[:, :],
                                                       ALU.mult, ALU.add)
                        h1 = epool.tile([P, DH], BF16, tag="h1")
                        nc.scalar.activation(h1[:, :], v[:, :], AF.Relu)
                        hT = []
                        for k in range(DH // P):
                            pt = ptpool1.tile([P, P], BF16, tag="pt")
                            nc.tensor.transpose(pt[:, :],
                                                h1[:, k * P:(k + 1) * P],
                                                ident[:, :])
                            hTk = epool.tile([P, P], BF16, tag=f"hT{k}",
                                             name=f"hT{k}_{w}")
                            nc.scalar.activation(hTk[:, :], pt[:, :], AF.Copy)
                            hT.append(hTk)
                        ps2 = pspool1.tile([P, DOUT], F32, tag="gemm2",
                                           name=f"gemm2_{w}")
                        for k in range(DH // P):
                            nc.tensor.matmul(ps2[:, :], hT[k][:, :],
                                             W2_sb[k][:, :],
                                             start=(k == 0),
                                             stop=(k == DH // P - 1))
                        nc.scalar.activation(g2_sb[:, w, :], ps2[:, :], AF.Copy,
                                             scale=dinv_sb[:, w:w + 1])
                        s = int(np.searchsorted(plan.SB, w * P, side="right")) - 1
                        nc.sync.dma_start(
                            g2_in[s][w * P - plan.SB[s]:
                                     (w + 1) * P - plan.SB[s], :],
                            g2_sb[:, w, :])

                def ag2(s):
                    def emit():
                        all_gather(g2_in[s], g2_tbl[s])
                    return emit

                # AG of g2 stripe s right after its last window's epilogue
                bounds = np.cumsum(SWIN) - 1
                agw = {int(bounds[s]): ag2(s) for s in range(NSW)}
                run_pass(1, g1_tbl, DIN, L1, paypools1, LOOKAHEAD, PAYBUFS,
                         agw)

            g1k.__exit__(None, None, None)

            # ---- layer 2 consume + normalize ---------------------------------
            with tc.tile_pool(name="ps2", bufs=4, space="PSUM") as pspool2:

                class L2:
                    @staticmethod
                    def psum(w):
                        return pspool2.tile([P, DOUT], F32, tag="agg2",
                                            name=f"agg2_{w}")

                    @staticmethod
                    def selfrows(w):
                        return g2_sb[:, w, :]

                    @staticmethod
                    def epilogue(w, ps):
                        v = epool.tile([P, DOUT], F32, tag="v2")
                        nc.vector.scalar_tensor_tensor(v[:, :], ps[:, :],
                                                       dinv_sb[:, w:w + 1],
                                                       b2_sb[:, :],
                                                       ALU.mult, ALU.add)
                        sq = epool.tile([P, DOUT], F32, tag="sq")
                        ss = epool.tile([P, 1], F32, tag="ss")
                        nc.scalar.activation(sq[:, :], v[:, :], AF.Square,
                                             accum_out=ss[:, :])
                        ssm = epool.tile([P, 1], F32, tag="ssm")
                        nc.vector.tensor_scalar_max(ssm[:, :], ss[:, :], 1e-24)
                        sr = epool.tile([P, 1], F32, tag="sr")
                        nc.scalar.activation(sr[:, :], ssm[:, :], AF.Sqrt)
                        inv = epool.tile([P, 1], F32, tag="inv")
                        nc.vector.reciprocal(inv[:, :], sr[:, :])
                        ot = epool.tile([P, DOUT], F32, tag="ot")
                        nc.scalar.activation(ot[:, :], v[:, :], AF.Copy,
                                             scale=inv[:, 0:1])
                        rows = min(P, NLOC - w * P)
                        nc.sync.dma_start(out[w * P:w * P + rows, :],
                                          ot[:rows, :])

                run_pass(2, g2_tbl, DOUT, L2, paypools2, LOOKAHEAD2, PAYBUFS2)

    nc.compile()
    return nc


_CACHE = {}


def kernel(x, edge_index, W1, b1, W2, b2, **_ignored):
    x = np.asarray(x)
    plan, in_maps = preprocess(x, edge_index, W1, b1, W2, b2)
    key = plan.key()
    if key not in _CACHE:
        _CACHE[key] = build(plan)
    nc = _CACHE[key]
    res = run_bass_kernel_spmd(nc, in_maps, core_ids=list(range(C)))
    return np.concatenate([res.results[c]["out"] for c in range(C)], axis=0)
